# revision 1
# baseline (speedup 1.0000x reference)
"""Trainium2 Bass kernel for the Evoformer block (nn_Evoformer_30365418782821).

Sharding: 8 cores = data-parallel over batch (B=2) x sequence-parallel over
the query axis (4 shards of 512). Each core computes its full [512, 128]
output slice with no collectives; host scatters inputs / gathers outputs.

Per-core dataflow (all activations transposed [C, rows], weights as matmul
stationary operands):
  - adaptive LN on k/q sides (bn_stats row-major, then PE transpose)
  - attention computed as S^T[k, q] in PSUM: QK^T via 4-way row-tiled
    matmuls (heads padded to 32-partition strips), pair_logits added via
    PE transpose-matmuls (lhsT=pair block, rhs=identity), exp on ACT
    (PSUM -> SBUF bf16), PV col-tiled with a ones-column in v producing
    softmax denominators for free
  - pair_logits streamed HBM->SBUF with SWDGE fp32->bf16 cast
  - sigmoid/rsqrt built from Exp/Ln only (single ACT table set)
"""

import numpy as np

B, N, C, H, CI = 2, 2048, 128, 8, 512
D = C // H
EPS = 1e-5
QS = 512          # query rows per core
NCORES = 8
KC = 16           # k chunks of 128
QB = 4            # q blocks of 128

_cached = {}


def _build(loop_n=1, parts="full"):
    import concourse.bacc as bacc
    import concourse.mybir as mybir
    import concourse.tile as tile
    from concourse.masks import make_identity

    f32 = mybir.dt.float32
    bf16 = mybir.dt.bfloat16
    AF = mybir.ActivationFunctionType
    AL = mybir.AluOpType

    import concourse.mybir as _mb

    class _OneTableBacc(bacc.Bacc):
        # Mask every ACT table set except the one holding Exp/Ln/Identity/
        # Copy/Square, so the greedy set chooser cannot thrash between
        # exp_and_others and natural_log (ids stay positional).
        def insert_act_table_loads(self):
            from concourse.hw_specs import get_activation_tables
            has_activation = any(
                isinstance(i, _mb.InstActivation)
                for b in self.main_func.blocks
                for i in b.instructions
            )
            if not has_activation:
                return
            tables = [
                (k, (v if k == "natural_log_exp_and_others" else set()))
                for k, v in get_activation_tables(self.m.arch).items()
            ]
            from concourse.bacc import _bass_rust as _br
            _br.insert_act_table_loads(self, tables)

    nc = _OneTableBacc("TRN2", target_bir_lowering=False)

    # ---- DRAM I/O ----
    xq_d = nc.dram_tensor("xq", [QS, C], f32, kind="ExternalInput")
    cq_d = nc.dram_tensor("cq", [QS, C], f32, kind="ExternalInput")
    xk_d = nc.dram_tensor("xk", [N, C], f32, kind="ExternalInput")
    ck_d = nc.dram_tensor("ck", [N, C], f32, kind="ExternalInput")
    pair_d = nc.dram_tensor("pair", [H, N, QS], f32, kind="ExternalInput")
    wn = [
        ("qsw", [C, C]), ("qsb", [C]), ("qbw", [C, C]), ("qcw", [C]),
        ("ksw", [C, C]), ("ksb", [C]), ("kbw", [C, C]), ("kcw", [C]),
        ("wq", [C, C]), ("bq", [C]), ("wk", [C, C]), ("wv", [C, C]),
        ("wg", [C, C]), ("azi_wt", [C, C]), ("azi_wc", [C, C]), ("azi_bc", [C]),
        ("tsw", [C, C]), ("tsb", [C]), ("tbw", [C, C]), ("tcw", [C]),
        ("glu1", [C, CI]), ("glu2", [C, CI]), ("tawt", [CI, C]),
        ("tawc", [C, C]), ("tabc", [C]),
    ]
    wd = {name: nc.dram_tensor(name, shape, f32, kind="ExternalInput")
          for name, shape in wn}
    y_d = nc.dram_tensor("y", [QS, C], f32, kind="ExternalOutput")

    with tile.TileContext(nc) as tc:
        with tc.tile_pool(name="consts", bufs=1) as cp, \
             tc.tile_pool(name="pers", bufs=1) as pp, \
             tc.tile_pool(name="pairp", bufs=2) as pairp:

            def body():
                # ======== constants ========
                ident32 = cp.tile([128, 128], f32, name="ident32")
                make_identity(nc, ident32)
                identbf = cp.tile([128, 128], bf16, name="identbf")
                make_identity(nc, identbf)
                ones_col = cp.tile([128, 1], f32, name="ones_col")
                nc.vector.memset(ones_col, 1.0)
                ones_row = cp.tile([1, 128], f32, name="ones_row")
                nc.vector.memset(ones_row, 1.0)
                eps_t = cp.tile([128, 1], f32, name="eps_t")
                nc.vector.memset(eps_t, EPS)
                Rsel = cp.tile([4, 128], f32, name="Rsel")
                nc.vector.memset(Rsel, 0.0)
                mask16 = cp.tile([1, 16], f32, name="mask16")
                nc.vector.memset(mask16, 1.0)
                for h in range(4):
                    nc.sync.dma_start(out=Rsel[h : h + 1, 32 * h : 32 * h + 16],
                                      in_=mask16)

                # ======== weights ========
                w = {}
                for name in ("qsw", "qbw", "ksw", "kbw", "tsw", "tbw",
                             "azi_wc", "tawc"):
                    t = cp.tile([128, 128], f32, name=name)
                    nc.sync.dma_start(out=t, in_=wd[name][:])
                    w[name] = t
                for name in ("glu1", "glu2"):
                    t = cp.tile([128, CI], f32, name=name)
                    nc.sync.dma_start(out=t, in_=wd[name][:])
                    w[name] = t
                tawt = cp.tile([128, 4, 128], f32, name="tawt")
                nc.sync.dma_start(out=tawt, in_=wd["tawt"].rearrange("(t p) c -> p t c", p=128))
                vecs = {}
                for name in ("qcw", "kcw", "tcw", "qsb", "ksb", "tsb",
                             "azi_bc", "tabc"):
                    t = cp.tile([128, 1], f32, name="v_" + name)
                    nc.sync.dma_start(out=t, in_=wd[name].rearrange("(c one) -> c one", one=1))
                    vecs[name] = t
                # fold cond weights into scale/bias weights (rows scaled)
                for cname, tnames in (("kcw", ("ksw", "kbw")),
                                      ("qcw", ("qsw", "qbw")),
                                      ("tcw", ("tsw", "tbw"))):
                    for tn in tnames:
                        nc.vector.tensor_scalar_mul(w[tn], w[tn], vecs[cname])
                # negated biases for sigmoid-via-exp
                for name in ("qsb", "ksb", "tsb", "azi_bc", "tabc"):
                    nc.vector.tensor_scalar_mul(vecs[name], vecs[name], -1.0)

                # padded weights: within group g, col/row 32h+d <- dense 64g+16h+d
                def pad_cols(src_ap, tag, ncols=128, groups=2):
                    tiles = []
                    for g in range(groups):
                        t = cp.tile([128, ncols], f32, name=f"{tag}{g}")
                        nc.vector.memset(t, 0.0)
                        dst = t.rearrange("c (h x) -> c h x", x=32)[:, :, :16]
                        src = src_ap[:, 64 * g : 64 * g + 64].rearrange(
                            "c (h d) -> c h d", d=16)
                        nc.sync.dma_start(out=dst, in_=src)
                        tiles.append(t)
                    return tiles

                wq_pad = pad_cols(wd["wq"], "wq_pad")
                wk_pad = pad_cols(wd["wk"], "wk_pad")
                wg_pad = pad_cols(wd["wg"], "wg_pad")
                # wv_pad [128, 256]: col 128g+32h+d
                wv_pad = cp.tile([128, 256], f32, name="wv_pad")
                nc.vector.memset(wv_pad, 0.0)
                nc.sync.dma_start(
                    out=wv_pad.rearrange("c (g h x) -> c g h x", g=2, x=32)[:, :, :, :16],
                    in_=wd["wv"].rearrange("c (g h d) -> c g h d", g=2, d=16),
                )
                # azi_wt_pad: padded rows
                azi_wt_pad = []
                for g in range(2):
                    t = cp.tile([128, 128], f32, name=f"azi_wt_pad{g}")
                    nc.vector.memset(t, 0.0)
                    dst = t.rearrange("(h x) c -> h x c", x=32)[:, :16, :]
                    src = wd["azi_wt"][64 * g : 64 * g + 64, :].rearrange(
                        "(h d) c -> h d c", d=16)
                    nc.sync.dma_start(out=dst, in_=src)
                    azi_wt_pad.append(t)
                # bq_pad [128,1] per group, pre-scaled by 0.25
                bq_pad = []
                for g in range(2):
                    t = cp.tile([128, 1], f32, name=f"bq_pad{g}")
                    nc.vector.memset(t, 0.0)
                    dst = t.rearrange("(h x) f -> h x f", x=32)[:, :16, :]
                    src = wd["bq"][64 * g : 64 * g + 64].rearrange("(h d one) -> h d one", d=16, one=1)
                    nc.sync.dma_start(out=dst, in_=src)
                    nc.vector.tensor_scalar_mul(t, t, 0.25)
                    bq_pad.append(t)

                # ======== pair DMAs (SWDGE cast fp32->bf16), quarter chunks ========
                # issued early in trace order; consumed in the attention loop
                pair_ap = pair_d.rearrange("h (j p) q -> h p j q", p=128)
                pair_tiles = [[None] * 4 for _ in range(H)]
                for jb in range(4):
                    for head in range(H):
                        t = pairp.tile([128, 4, 512], bf16, name=f"pair{head}")
                        nc.gpsimd.dma_start(
                            out=t, in_=pair_ap[head][:, 4 * jb : 4 * jb + 4, :])
                        pair_tiles[head][jb] = t

                if parts == "dma":
                    with tc.tile_pool(name="dacc", bufs=1) as dac:
                        acc = dac.tile([128, 32], f32, name="dacc_t")
                        for jb in range(4):
                            for head in range(H):
                                nc.vector.tensor_copy(
                                    out=acc[:, 4 * jb + head // 2 : 4 * jb + head // 2 + 1],
                                    in_=pair_tiles[head][jb][:, 0, 0:1])
                        nc.sync.dma_start(
                            out=y_d.rearrange("(i p) c -> p i c", p=128)[:, 0, 0:32],
                            in_=acc)
                    return

                # ======== prep: k-side then q-side ========
                def sigmoid_from_psum(out_sb, ps, neg_bias):
                    # out = 1/(1+exp(-(ps + bias)));  exp part on ACT, rest on DVE
                    nc.scalar.activation(out_sb, ps, AF.Exp, bias=neg_bias, scale=-1.0)
                    nc.vector.tensor_scalar_add(out_sb, out_sb, 1.0)
                    nc.vector.reciprocal_approx_fast(out=out_sb, in_=out_sb)

                with tc.tile_pool(name="prep", bufs=1) as prp, \
                     tc.tile_pool(name="prept", bufs=3) as prt, \
                     tc.tile_pool(name="ppsum", bufs=2, space="PSUM") as pps:

                    def ln_rows_to_T(x_dram, nrows, tagbase):
                        """DMA row-major [nrows, C], LN over C, transpose ->
                        returns [128, nrows] fp32 tile (transposed, normalized)."""
                        nt = nrows // 128
                        outT = prp.tile([128, nrows], f32, name=f"{tagbase}T")
                        x_r = x_dram.rearrange("(t p) c -> p t c", p=128)
                        for b4 in range(nt // 4):
                            rows = prt.tile([128, 4, 128], f32, name=f"{tagbase}_rows")
                            nc.sync.dma_start(out=rows, in_=x_r[:, 4 * b4 : 4 * b4 + 4, :])
                            ps = pps.tile([128, 4, 128], f32, name="tps")
                            mv = prt.tile([128, 4, 2], f32, name="mv4", tag="mv4")
                            for t in range(4):
                                st = prt.tile([128, 6], f32, name="st", tag="st")
                                nc.vector.bn_stats(st, rows[:, t, :])
                                nc.vector.bn_aggr(mv[:, t, :], st)
                            rstd = prt.tile([128, 4], f32, name="rstd4", tag="rstd4")
                            nc.scalar.activation(rstd, mv[:, :, 1], AF.Ln,
                                                 bias=eps_t)
                            nc.scalar.activation(rstd, rstd, AF.Exp, scale=-0.5)
                            for t in range(4):
                                nc.vector.tensor_scalar(
                                    rows[:, t, :], rows[:, t, :],
                                    scalar1=mv[:, t, 0:1],
                                    scalar2=rstd[:, t : t + 1],
                                    op0=AL.subtract, op1=AL.mult)
                                nc.tensor.matmul(ps[:, t, :], lhsT=rows[:, t, :],
                                                 rhs=ident32)
                            nc.vector.tensor_copy(
                                out=outT[:, 512 * b4 : 512 * b4 + 512],
                                in_=ps.rearrange("p t c -> p (t c)"))
                        return outT

                    def raw_T(x_dram, nrows, tagbase):
                        """transpose raw rows without LN."""
                        nt = nrows // 128
                        outT = pp.tile([128, nrows], f32, name=f"{tagbase}T")
                        x_r = x_dram.rearrange("(t p) c -> p t c", p=128)
                        for b4 in range(nt // 4):
                            rows = prt.tile([128, 4, 128], f32, name=f"{tagbase}_rows")
                            nc.sync.dma_start(out=rows, in_=x_r[:, 4 * b4 : 4 * b4 + 4, :])
                            ps = pps.tile([128, 4, 128], f32, name="tps")
                            for t in range(4):
                                nc.tensor.matmul(ps[:, t, :], lhsT=rows[:, t, :],
                                                 rhs=ident32)
                            nc.vector.tensor_copy(
                                out=outT[:, 512 * b4 : 512 * b4 + 512],
                                in_=ps.rearrange("p t c -> p (t c)"))
                        return outT

                    # ---- k side ----
                    if parts == "attn":
                        kT_pad = [pp.tile([128, N], bf16, name=f"kT_pad{g}")
                                  for g in range(2)]
                        qT_pad = [pp.tile([128, QS], bf16, name=f"qT_pad{g}")
                                  for g in range(2)]
                        gate_padT = [pp.tile([128, QS], f32, name=f"gate{g}")
                                     for g in range(2)]
                        v_sb = [pp.tile([128, 256], bf16, name=f"v{j}")
                                for j in range(KC)]
                        for t in kT_pad + qT_pad + gate_padT + v_sb:
                            nc.vector.memset(t, 0.0)
                    else:
                        xknT = ln_rows_to_T(xk_d, N, "xkn")
                        cknT = ln_rows_to_T(ck_d, N, "ckn")
                        xk_adaT = pp.tile([128, N], f32, name="xk_adaT")
                        for ch in range(4):
                            sl = slice(512 * ch, 512 * ch + 512)
                            ps = pps.tile([128, 512], f32, name="kps")
                            nc.tensor.matmul(ps, lhsT=w["ksw"], rhs=cknT[:, sl])
                            sig = prt.tile([128, 512], f32, name="ksig")
                            sigmoid_from_psum(sig, ps, vecs["ksb"])
                            ps2 = pps.tile([128, 512], f32, name="kps2")
                            nc.tensor.matmul(ps2, lhsT=w["kbw"], rhs=cknT[:, sl])
                            nc.vector.tensor_tensor(xk_adaT[:, sl], sig, xknT[:, sl], AL.mult)
                            nc.vector.tensor_tensor(xk_adaT[:, sl], xk_adaT[:, sl], ps2, AL.add)

                        # kT_pad (bf16) and v tiles
                        kT_pad = [pp.tile([128, N], bf16, name=f"kT_pad{g}") for g in range(2)]
                        for g in range(2):
                            for ch in range(4):
                                sl = slice(512 * ch, 512 * ch + 512)
                                ps = pps.tile([128, 512], f32, name="kps")
                                nc.tensor.matmul(ps, lhsT=wk_pad[g], rhs=xk_adaT[:, sl])
                                nc.vector.tensor_copy(out=kT_pad[g][:, sl], in_=ps)
                        v_sb = []
                        for j in range(KC):
                            ps = pps.tile([128, 256], f32, name="vps")
                            nc.tensor.matmul(ps, lhsT=xk_adaT[:, 128 * j : 128 * j + 128],
                                             rhs=wv_pad)
                            vt = pp.tile([128, 256], bf16, name=f"v{j}")
                            nc.vector.tensor_copy(out=vt, in_=ps)
                            nc.vector.memset(
                                vt.rearrange("p (G x) -> p G x", x=32)[:, :, 16], 1.0)
                            v_sb.append(vt)

                        # ---- q side ----
                        xqnT = ln_rows_to_T(xq_d, QS, "xqn")
                        cqnT_l = ln_rows_to_T(cq_d, QS, "cqn")
                        cqnT = pp.tile([128, QS], f32, name="cqnT")
                        nc.vector.tensor_copy(out=cqnT, in_=cqnT_l)
                        cqT_raw = raw_T(cq_d, QS, "cq_raw")
                        xqT_raw = raw_T(xq_d, QS, "xq_raw")

                        ps = pps.tile([128, 512], f32, name="kps")
                        nc.tensor.matmul(ps, lhsT=w["qsw"], rhs=cqnT_l)
                        sigq = prt.tile([128, 512], f32, name="qsig")
                        sigmoid_from_psum(sigq, ps, vecs["qsb"])
                        ps2 = pps.tile([128, 512], f32, name="kps2")
                        nc.tensor.matmul(ps2, lhsT=w["qbw"], rhs=cqnT_l)
                        xq_adaT = prp.tile([128, QS], f32, name="xq_adaT")
                        nc.vector.tensor_tensor(xq_adaT, sigq, xqnT, AL.mult)
                        nc.vector.tensor_tensor(xq_adaT, xq_adaT, ps2, AL.add)

                        qT_pad, gate_padT = [], []
                        for g in range(2):
                            ps = pps.tile([128, 512], f32, name="kps")
                            nc.tensor.matmul(ps, lhsT=wq_pad[g], rhs=xq_adaT)
                            qt = pp.tile([128, QS], bf16, name=f"qT_pad{g}")
                            nc.scalar.activation(qt, ps, AF.Identity,
                                                 bias=bq_pad[g], scale=0.25)
                            qT_pad.append(qt)
                            ps2 = pps.tile([128, 512], f32, name="kps2")
                            nc.tensor.matmul(ps2, lhsT=wg_pad[g], rhs=xq_adaT)
                            gt = pp.tile([128, QS], f32, name=f"gate{g}")
                            sigmoid_from_psum(gt, ps2, 0.0)
                            gate_padT.append(gt)

                        # gates that depend only on inputs (computed in prep)
                        azigT = pp.tile([128, QS], f32, name="azigT")
                        ps = pps.tile([128, 512], f32, name="kps")
                        nc.tensor.matmul(ps, lhsT=w["azi_wc"], rhs=cqT_raw)
                        sigmoid_from_psum(azigT, ps, vecs["azi_bc"])
                        tgT = pp.tile([128, QS], f32, name="tgT")
                        ps = pps.tile([128, 512], f32, name="kps")
                        nc.tensor.matmul(ps, lhsT=w["tawc"], rhs=cqT_raw)
                        sigmoid_from_psum(tgT, ps, vecs["tabc"])
                        tsigT = pp.tile([128, QS], f32, name="tsigT")
                        ps = pps.tile([128, 512], f32, name="kps")
                        nc.tensor.matmul(ps, lhsT=w["tsw"], rhs=cqnT)
                        sigmoid_from_psum(tsigT, ps, vecs["tsb"])
                        tbiasT = pp.tile([128, QS], f32, name="tbiasT")
                        ps = pps.tile([128, 512], f32, name="kps")
                        nc.tensor.matmul(ps, lhsT=w["tbw"], rhs=cqnT)
                        nc.vector.tensor_copy(out=tbiasT, in_=ps)

                # ======== attention ========
                og = []
                with tc.tile_pool(name="ep", bufs=5) as ep, \
                     tc.tile_pool(name="epi", bufs=1) as tr, \
                     tc.tile_pool(name="psS", bufs=3, space="PSUM") as psS, \
                     tc.tile_pool(name="pout", bufs=1, space="PSUM") as pout:
                    out_ps = [pout.tile([128, QS], f32, name=f"out{g}") for g in range(2)]
                    pending = []  # deferred PV ops: (g, j, h, E)
                    def flush_pv():
                        for (pg, pj, ph, pE) in pending:
                            nc.tensor.matmul(
                                out_ps[pg][32 * ph : 32 * ph + 32, :],
                                lhsT=v_sb[pj][:, 128 * pg + 32 * ph : 128 * pg + 32 * ph + 32],
                                rhs=pE,
                                start=(pj == 0), stop=(pj == KC - 1),
                                tile_position=(0, 32 * ph))
                        pending.clear()

                    for jb in range(4):
                        for g in range(2):
                            for dj in range(4):
                                j = 4 * jb + dj
                                S2s = []
                                for hp in range(2):
                                    S2 = psS.tile([128, 2, QS], f32, name="S2", tag="S")
                                    for i in range(2):
                                        h = 2 * hp + i
                                        head = 4 * g + h
                                        rows = slice(32 * h, 32 * h + 32)
                                        nc.tensor.matmul(
                                            S2[:, i, :],
                                            lhsT=kT_pad[g][rows, 128 * j : 128 * j + 128],
                                            rhs=qT_pad[g][rows, :],
                                            start=True, stop=False,
                                            tile_position=(32 * h, 0))
                                        pq = pair_tiles[head][jb]
                                        nc.tensor.matmul(
                                            S2[:, i, :], lhsT=identbf, rhs=pq[:, dj, :],
                                            start=False, stop=True,
                                            tile_position=(0, 0))
                                    S2s.append(S2)
                                flush_pv()
                                for hp in range(2):
                                    E2 = ep.tile([128, 2, QS], bf16, name="E", tag="E")
                                    nc.scalar.activation(E2, S2s[hp], AF.Exp)
                                    for i in range(2):
                                        pending.append((g, j, 2 * hp + i, E2[:, i, :]))
                    flush_pv()

                    if parts == "attn":
                        ab = ep.tile([128, QS], f32, name="ab", tag="E")
                        nc.vector.tensor_copy(out=ab, in_=out_ps[0])
                        nc.sync.dma_start(
                            out=y_d.rearrange("(i p) c -> p i c", p=128), in_=ab.rearrange("p (i c) -> p i c", c=128))
                        return

                    # ---- epilogue: normalize, gate, azi, residual ----
                    yT = pp.tile([128, QS], f32, name="yT")
                    ps_o = psS.tile([128, QS], f32, name="ps_o", tag="S")
                    for g in range(2):
                        out_sb = tr.tile([128, QS], f32, name=f"outsb{g}")
                        nc.vector.tensor_copy(out=out_sb, in_=out_ps[g])
                        dn = tr.tile([4, QS], f32, name="dn")
                        nc.sync.dma_start(
                            out=dn,
                            in_=out_sb.rearrange("(h x) q -> h x q", x=32)[:, 16, :])
                        nc.vector.reciprocal_approx_fast(out=dn, in_=dn)
                        ps_r = psS.tile([128, QS], f32, name="ps_r", tag="S")
                        nc.tensor.matmul(ps_r, lhsT=Rsel, rhs=dn)
                        o = tr.tile([128, QS], f32, name=f"og{g}")
                        nc.vector.tensor_tensor(o, out_sb, ps_r, AL.mult)
                        nc.vector.tensor_tensor(o, o, gate_padT[g], AL.mult)
                        og.append(o)
                    nc.tensor.matmul(ps_o, lhsT=azi_wt_pad[0], rhs=og[0],
                                     start=True, stop=False)
                    nc.tensor.matmul(ps_o, lhsT=azi_wt_pad[1], rhs=og[1],
                                     start=False, stop=True)
                    nc.vector.tensor_tensor(yT, ps_o, azigT, AL.mult)
                    nc.vector.tensor_tensor(yT, yT, xqT_raw, AL.add)

                # ======== transition ========
                with tc.tile_pool(name="tr1", bufs=1) as tr, \
                     tc.tile_pool(name="trs", bufs=4) as trs, \
                     tc.tile_pool(name="tpsum", bufs=1, space="PSUM") as tps, \
                     tc.tile_pool(name="tpsum2", bufs=2, space="PSUM") as tps2:
                    ysq = trs.tile([128, QS], f32, name="ysq", tag="scratch")
                    nc.vector.tensor_tensor(ysq, yT, yT, AL.mult)
                    ps_s1 = tps.tile([1, QS], f32, name="s1")
                    nc.tensor.matmul(ps_s1, lhsT=ones_col, rhs=yT)
                    ps_s2 = tps.tile([1, QS], f32, name="s2", tag="s1")
                    nc.tensor.matmul(ps_s2, lhsT=ones_col, rhs=ysq)
                    mean = tr.tile([1, QS], f32, name="mean")
                    nc.vector.tensor_copy(out=mean, in_=ps_s1)
                    nc.vector.tensor_scalar_mul(mean, mean, 1.0 / 128.0)
                    var = tr.tile([1, QS], f32, name="var")
                    nc.vector.tensor_copy(out=var, in_=ps_s2)
                    nc.vector.tensor_scalar_mul(var, var, 1.0 / 128.0)
                    m2 = tr.tile([1, QS], f32, name="m2")
                    nc.vector.tensor_tensor(m2, mean, mean, AL.mult)
                    nc.vector.tensor_tensor(var, var, m2, AL.subtract)
                    rstd = tr.tile([1, QS], f32, name="rstd")
                    nc.scalar.activation(rstd, var, AF.Ln, bias=eps_t[0:1, :])
                    nc.scalar.activation(rstd, rstd, AF.Exp, scale=-0.5)
                    nmr = tr.tile([1, QS], f32, name="nmr")
                    nc.vector.tensor_tensor(nmr, mean, rstd, AL.mult)
                    nc.vector.tensor_scalar_mul(nmr, nmr, -1.0)
                    ps_a = tps.tile([128, QS], f32, name="ps_a", tag="ps_a")
                    nc.tensor.matmul(ps_a, lhsT=ones_row, rhs=rstd)
                    ps_b = tps.tile([128, QS], f32, name="ps_b")
                    nc.tensor.matmul(ps_b, lhsT=ones_row, rhs=nmr)
                    yn = trs.tile([128, QS], f32, name="yn", tag="scratch")
                    nc.vector.tensor_tensor(yn, ps_a, yT, AL.mult)
                    nc.vector.tensor_tensor(yn, yn, ps_b, AL.add)
                    aT = tr.tile([128, QS], f32, name="aT")
                    nc.vector.tensor_tensor(aT, tsigT, yn, AL.mult)
                    nc.vector.tensor_tensor(aT, aT, tbiasT, AL.add)

                    ps_t = tps.tile([128, QS], f32, name="ps_t")
                    for t in range(4):
                        cs = slice(128 * t, 128 * t + 128)
                        ps1 = tps2.tile([128, QS], f32, name="ps1", tag="ps1")
                        nc.tensor.matmul(ps1, lhsT=w["glu1"][:, cs], rhs=aT)
                        e = trs.tile([128, QS], f32, name="sil_e", tag="scratch")
                        nc.scalar.activation(e, ps1, AF.Exp, scale=-1.0)
                        nc.vector.tensor_scalar_add(e, e, 1.0)
                        nc.vector.reciprocal_approx_fast(out=e, in_=e)
                        sil = trs.tile([128, QS], f32, name="sil", tag="scratch")
                        nc.vector.tensor_tensor(sil, e, ps1, AL.mult)
                        ps2 = tps2.tile([128, QS], f32, name="ps2", tag="ps2")
                        nc.tensor.matmul(ps2, lhsT=w["glu2"][:, cs], rhs=aT)
                        hh = trs.tile([128, QS], f32, name="hh", tag="scratch")
                        nc.vector.tensor_tensor(hh, sil, ps2, AL.mult)
                        nc.tensor.matmul(ps_t, lhsT=tawt[:, t, :], rhs=hh,
                                         start=(t == 0), stop=(t == 3))
                    youtT = trs.tile([128, QS], f32, name="youtT", tag="scratch")
                    nc.vector.tensor_tensor(youtT, ps_t, tgT, AL.mult)
                    nc.vector.tensor_tensor(youtT, youtT, yT, AL.add)

                    # un-transpose and write out
                    ps_y = tps.tile([128, 4, 128], f32, name="ps_y", tag="ps_a")
                    for i in range(4):
                        nc.tensor.matmul(ps_y[:, i, :],
                                         lhsT=youtT[:, 128 * i : 128 * i + 128],
                                         rhs=ident32)
                    yout = trs.tile([128, 4, 128], f32, name="yout", tag="scratch")
                    nc.vector.tensor_copy(out=yout, in_=ps_y)
                    nc.sync.dma_start(
                        out=y_d.rearrange("(i p) c -> p i c", p=128), in_=yout)

            if loop_n > 1:
                with tc.For_i(0, loop_n, 1):
                    body()
            else:
                body()

    nc.finalize()
    return nc


def _get_nc(loop_n=1, parts="full"):
    key = (loop_n, parts)
    if key not in _cached:
        _cached[key] = _build(loop_n, parts)
    return _cached[key]


def make_in_maps(inputs):
    inputs = {k: np.ascontiguousarray(np.asarray(v), dtype=np.float32)
              for k, v in inputs.items()}
    ren = {
        "qsw": "q_ln_scale_w", "qsb": "q_ln_scale_b", "qbw": "q_ln_bias_w",
        "qcw": "q_ln_cond_w", "ksw": "k_ln_scale_w", "ksb": "k_ln_scale_b",
        "kbw": "k_ln_bias_w", "kcw": "k_ln_cond_w", "wq": "wq", "bq": "bq",
        "wk": "wk", "wv": "wv", "wg": "wg", "azi_wt": "azi_wt",
        "azi_wc": "azi_wc", "azi_bc": "azi_bc", "tsw": "t_ln_scale_w",
        "tsb": "t_ln_scale_b", "tbw": "t_ln_bias_w", "tcw": "t_ln_cond_w",
        "glu1": "glu1_w", "glu2": "glu2_w", "tawt": "t_azi_wt",
        "tawc": "t_azi_wc", "tabc": "t_azi_bc",
    }
    in_maps = []
    for core in range(NCORES):
        b, s = core // 4, core % 4
        q0 = s * QS
        m = {
            "xq": inputs["x_q"][b, q0 : q0 + QS],
            "cq": inputs["single_cond_q"][b, q0 : q0 + QS],
            "xk": inputs["x_k"][b],
            "ck": inputs["single_cond_k"][b],
            "pair": inputs["pair_logits"][b, :, q0 : q0 + QS, :],
        }
        for short, full in ren.items():
            m[short] = inputs[full]
        in_maps.append({k: np.ascontiguousarray(v) for k, v in m.items()})
    return in_maps


def kernel(**inputs) -> np.ndarray:
    from concourse.bass_utils import run_bass_kernel_spmd

    nc = _get_nc()
    in_maps = make_in_maps(inputs)
    res = run_bass_kernel_spmd(nc, in_maps, core_ids=list(range(NCORES)))
    y = np.zeros((B, N, C), np.float32)
    for core in range(NCORES):
        b, s = core // 4, core % 4
        y[b, s * QS : (s + 1) * QS] = res.results[core]["y"]
    return y

